# revision 28
# baseline (speedup 1.0000x reference)
"""Trainium2 Bass kernel for nn_ACMAttention (Chan-Vese PDE attention block).

Self-contained: shards batch B=8 across 8 NeuronCores (pure data parallel).
Per core: 1x1-conv GEMMs on the TensorEngine, then a 10-iteration Chan-Vese
PDE loop in fp16 split across DVE (diffs, customs) and GPSIMD (normalized
gradient + divergence) with the Activation engine doing arctan + mirror
copies.  All big ops are emitted as row-halves so the two engines wavefront
within each iteration; the next tile's GEMM chunks are injected into the
current tile's PDE loop to avoid inter-tile bubbles.

Shapes (hardcoded, per core = one batch element):
  g [256,4096], x [512,4096], contour [256,4096], weights transposed on host.
"""
import sys
import numpy as np

for _p in ("/opt/trn_rl_repo",):
    if _p not in sys.path:
        sys.path.insert(0, _p)

from contextlib import ExitStack

import concourse.bass as bass
import concourse.bacc as bacc
import concourse.tile as tile
from concourse import mybir
from concourse import dve_ops as dvo
from concourse.dve_spec import (
    Spec, Src0, Src1, C0, C1, C2, Bin, AluOp, sq, maxx, minn, lower,
    _has_src1, select, Zero, One,
)
from concourse.dve_uop import DveOpSpec
from concourse import tile_utils

tile_utils.max_sbuf_usage = 204 * 1024

F32 = mybir.dt.float32
BF16 = mybir.dt.bfloat16
FP16 = mybir.dt.float16
AF = mybir.ActivationFunctionType
OP = mybir.AluOpType

NITER = 10
LINEARIZE = False
EPS4 = 4e-8           # 4 * 1e-8 (unscaled-gradient eps)
CHEB0 = -0.23549792   # recip NOT-seed Chebyshev pair
CHEB1 = 2.0017324

REFRESH_EVERY = 2      # refresh Yhat (rsqrt |grad|) on iters 0, k, 2k, ...
STATS_EVERY_ITER = False  # lag-1 stats every iter; False -> every other iter

HS = [(0, 32), (32, 64)]  # row-halves for wavefronting

# ---------------------------------------------------------------- custom ops

_REG = {}


def _register(name, body, reference, accum=False):
    if name in _REG:
        return _REG[name]
    row = max(dvo._SUB_OPCODE_FOR_NAME.values()) + 1
    assert row < 0x20
    dvo._SUB_OPCODE_FOR_NAME[name] = row
    if accum:
        from operator import add as _add
        spec = Spec(body=body, reference=reference, accum=_add,
                    accum_init=Zero)
    else:
        spec = Spec(body=body, reference=reference)
    shas = {}
    for ver in ("v3", "v4"):
        try:
            uops = lower(spec, ver=ver)
            shas[ver] = DveOpSpec(
                name=name, opcode=row, uops=uops, rd1_en=_has_src1(spec)
            ).sha(ver)
        except Exception:
            pass
    assert shas, f"custom op {name} failed to lower"
    op = dvo.DveOp(name, spec, subdim=False, uops_sha=shas)
    dvo.OPS.append(op)
    dvo.CUSTOM_DVE_SPECS[name] = spec
    _REG[name] = op
    return op


def _ref_sq2(in0, in1, c0, c1, c2):
    i0 = in0.astype(np.float32)
    i1 = in1.astype(np.float32).reshape(i0.shape)
    return (i0 ** 2 + i1 ** 2 + c0).astype(np.float32)


def _ref_rsqrt_nr(in0, in1, c0, c1, c2):
    i0 = in0.astype(np.float32)
    i1 = in1.astype(np.float32).reshape(i0.shape)
    m = np.maximum(np.float32(c0) - i0 * i1 * i1, np.float32(c1))
    return (i1 * m).astype(np.float32)


def _ref_atan_arg(in0, in1, c0, c1, c2):
    # U = phi * min(c2, recip1(phi^2));  DVE min drops NaN (recip1(0)=NaN -> c2)
    x = np.ascontiguousarray(in0.astype(np.float32))
    v = np.ascontiguousarray(x * x)
    nx = (~v.view(np.int32)).view(np.float32)
    y0 = nx * np.float32(c0)
    y1 = y0 * (np.float32(c1) - v * y0)
    m = np.where(np.isnan(y1), np.float32(c2), np.minimum(np.float32(c2), y1))
    return (x * m).astype(np.float32)


def _ref_atan_fix(in0, in1, c0, c1, c2):
    # in0 = phi, in1 = Ta ; out = full-range atan; accum = sum(out)
    # sp = (phi & c1) | c2 == copysign(pi/2, phi) with c1=-0.0, c2=pi/2
    phi = np.ascontiguousarray(in0.astype(np.float32))
    ta = in1.astype(np.float32).reshape(phi.shape)
    sp = ((phi.view(np.int32) & np.float32(c1).view(np.int32))
          | np.float32(c2).view(np.int32)).view(np.float32)
    b = np.where(phi * phi > np.float32(c0), sp - ta, ta).astype(np.float32)
    return b, b.reshape(b.shape[0], -1).sum(axis=-1, keepdims=True)


def _ref_esum(in0, in1, c0, c1, c2):
    i0 = in0.astype(np.float32)
    i1 = in1.astype(np.float32).reshape(i0.shape)
    return ((np.abs(i0) + np.abs(i1)) * np.float32(c0) + np.float32(c1)
            ).astype(np.float32)


def _ref_recipf(in0, in1, c0, c1, c2):
    # fp16-safe clone of RECIPROCAL_APPROX_FAST (cast before the bit trick)
    x = np.ascontiguousarray(in0.astype(np.float32))
    nx = (~x.view(np.int32)).view(np.float32)
    y0 = nx * np.float32(c0)
    y1 = y0 * (np.float32(c1) - x * y0)
    return (y1 * (np.float32(c2) - x * y1)).astype(np.float32)


def _ref_rff(in0, in1, c0, c1, c2):
    # out = cheb_recip(phi^2 + 1) * ff  with the Chebyshev pair (c0, c1)
    # pre-scaled by sqrt(kappa) so out = kappa * recip(phi^2+1) * ff
    phi = np.ascontiguousarray(in0.astype(np.float32))
    ff = in1.astype(np.float32).reshape(phi.shape)
    v = np.ascontiguousarray(phi * phi + np.float32(1.0))
    nx = (~v.view(np.int32)).view(np.float32)
    y0 = nx * np.float32(c0)
    y1 = y0 * (np.float32(c1) - v * y0)
    return (y1 * ff).astype(np.float32)


def _ref_reluadd(in0, in1, c0, c1, c2):
    i0 = in0.astype(np.float32)
    i1 = in1.astype(np.float32).reshape(i0.shape)
    return (np.maximum(i0, np.float32(0.0)) + i1).astype(np.float32)


def get_ops():
    sq2 = _register("ACM_SQ2ADDC", sq(Src0) + sq(Src1) + C0, _ref_sq2)
    nr = _register(
        "ACM_RSQRT_NR",
        Src1 * maxx(C0 - Src0 * sq(Src1), C1),
        _ref_rsqrt_nr,
    )
    _a0 = maxx(Src0, Zero - Src0)
    _a1 = maxx(Src1, Zero - Src1)
    esum = _register("ACM_ESUM", (_a0 + _a1) * C0 + C1, _ref_esum)
    _v2 = sq(Src0)
    _n = Bin(AluOp.BITWISE_NOT, _v2, _v2)
    _u0 = _n * C0
    _u1 = _u0 * (C1 - _v2 * _u0)
    uarg = _register("ACM_ATAN_ARG", Src0 * minn(C2, _u1), _ref_atan_arg)
    _sp = Bin(AluOp.BITWISE_OR, Bin(AluOp.BITWISE_AND, Src0, C1), C2)
    _tbody = select(sq(Src0) > C0, _sp - Src1, Src1)
    fix = _register("ACM_ATAN_FIX", _tbody, _ref_atan_fix, accum=True)
    _rnx = Bin(AluOp.BITWISE_NOT, Src0, Src0)
    _ry0 = _rnx * C0
    _ry1 = _ry0 * (C1 - Src0 * _ry0)
    recipf = _register("ACM_RECIPF", _ry1 * (C2 - Src0 * _ry1), _ref_recipf)
    _v = sq(Src0) + One
    _nx = Bin(AluOp.BITWISE_NOT, _v, _v)
    _y0 = _nx * C0
    _y1 = _y0 * (C1 - _v * _y0)
    rff = _register("ACM_RFF", _y1 * Src1, _ref_rff)
    reluadd = _register("ACM_RELUADD", maxx(Src0, Zero) + Src1, _ref_reluadd)
    return sq2, nr, esum, uarg, fix, recipf, rff, reluadd


# ---------------------------------------------------------------- builder

def build(dt_val: float, lam_val: float) -> bass.Bass:
    (sq2_op, nr_op, esum_op, uarg_op, fix_op, recipf_op, rff_op,
     reluadd_op) = get_ops()

    nc = bacc.Bacc()
    g_d = nc.declare_dram_parameter("g", [256, 4096], BF16, isOutput=False)
    x_d = nc.declare_dram_parameter("x", [512, 4096], BF16, isOutput=False)
    c_d = nc.declare_dram_parameter("contour", [256, 4096], BF16, isOutput=False)
    gw_d = nc.declare_dram_parameter("g_wT", [256, 512], BF16, isOutput=False)
    xw_d = nc.declare_dram_parameter("x_wT", [512, 512], BF16, isOutput=False)
    cw_d = nc.declare_dram_parameter("c_wT", [256, 512], BF16, isOutput=False)
    out_d = nc.declare_dram_parameter("out", [512, 4096], FP16, isOutput=True)

    # kappa = 1/s_r = dt/(pi*sqrt(2)); folded into the RFF Chebyshev pair
    # (scaled by sqrt(kappa)) so G = kappa * recip(phi^2+1) * FF in one op
    KAP = float(dt_val / (np.pi * np.sqrt(2.0)))
    RK0 = float(CHEB0 * np.sqrt(KAP))
    RK1 = float(CHEB1 * np.sqrt(KAP))
    A_SC = float(2.0 * np.sqrt(2.0) * lam_val)
    INV_PI = float(1.0 / np.pi)
    HPI = float(np.pi / 2)

    with tile.TileContext(nc, linearize=LINEARIZE) as tc, ExitStack() as ctx:
        v = nc.vector
        s = nc.scalar
        gp = nc.gpsimd

        wpool = ctx.enter_context(tc.tile_pool(name="w", bufs=1))
        inpool = ctx.enter_context(tc.tile_pool(name="in", bufs=2))
        psum = ctx.enter_context(tc.tile_pool(name="ps", bufs=2, space="PSUM"))
        state = ctx.enter_context(tc.tile_pool(name="st", bufs=2))
        tmp = ctx.enter_context(tc.tile_pool(name="tmp", bufs=1))
        sm = ctx.enter_context(tc.tile_pool(name="sm", bufs=2))

        # ---- load weights once (bf16, converted host-side): lhsT [k, o]
        gw_b = wpool.tile([128, 2, 512], BF16, tag="gwb", name="gwb")
        xw_b = wpool.tile([128, 4, 512], BF16, tag="xwb", name="xwb")
        cw_b = wpool.tile([128, 2, 512], BF16, tag="cwb", name="cwb")
        nc.sync.dma_start(out=gw_b, in_=gw_d.rearrange("(k p) m -> p k m", p=128))
        nc.sync.dma_start(out=xw_b, in_=xw_d.rearrange("(k p) m -> p k m", p=128))
        nc.sync.dma_start(out=cw_b, in_=cw_d.rearrange("(k p) m -> p k m", p=128))
        gw_t = [gw_b[:, k, :] for k in range(2)]
        xw_t = [xw_b[:, k, :] for k in range(4)]
        cw_t = [cw_b[:, k, :] for k in range(2)]
        # absorber matmuls: PE observes the weight DMA ticks once
        pdum = psum.tile([1, 1], F32, tag="pdum", name="pdum", bufs=1)
        nc.tensor.matmul(out=pdum, lhsT=gw_b[:, 0, 0:1], rhs=gw_b[:, 0, 0:1],
                         start=True, stop=True)
        nc.tensor.matmul(out=pdum, lhsT=xw_b[:, 0, 0:1], rhs=xw_b[:, 0, 0:1],
                         start=True, stop=True)
        nc.tensor.matmul(out=pdum, lhsT=cw_b[:, 0, 0:1], rhs=cw_b[:, 0, 0:1],
                         start=True, stop=True)

        g_r = g_d.rearrange("(k p) m -> p k m", p=128)
        x_r = x_d.rearrange("(k p) m -> p k m", p=128)
        c_r = c_d.rearrange("(k p) m -> p k m", p=128)

        def ghost_edge(eng, dst, a0, a1):
            # dst = 2*a0 - a1 (ghost extrapolation == one-sided edge diff)
            eng.scalar_tensor_tensor(out=dst, in0=a0, scalar=2.0, in1=a1,
                                     op0=OP.mult, op1=OP.subtract)

        def ghost_edge_pool(dst, a0, a1, scr):
            # Pool lacks TensorScalarPtr on HW: 2*a0 - a1 as two TTs
            gp.tensor_tensor(out=scr, in0=a0, in1=a1, op=OP.subtract)
            gp.tensor_tensor(out=dst, in0=a0, in1=scr, op=OP.add)

        st = {}

        def start_tile(t):
            p = t % 2
            d = {}
            d["PHI"] = state.tile([128, 64, 64], FP16, tag=f"phi{p}",
                                  name=f"phi_{t}")
            d["PHIB"] = state.tile([128, 66, 66], FP16, tag=f"phib{p}",
                                   name=f"phib_{t}", bufs=1)
            d["I"] = state.tile([128, 64, 64], FP16, tag=f"ii{p}",
                                name=f"ii_{t}", bufs=1)
            SIT2 = sm.tile([128, 2], F32, tag=f"sI{p}", name=f"sI_{t}")
            d["sI"] = SIT2[:, 0:1]
            d["halfsI"] = SIT2[:, 1:2]
            d["Yhat"] = None
            st[t] = d

        def gemm_chunk(t, n):
            d = st[t]
            osl = slice(t * 128, (t + 1) * 128)
            nsl = slice(n * 512, (n + 1) * 512)
            gch = inpool.tile([128, 2, 512], BF16, tag="gch", name="gch")
            xch = inpool.tile([128, 4, 512], BF16, tag="xch", name="xch")
            cch = inpool.tile([128, 2, 512], BF16, tag="cch", name="cch")
            nc.sync.dma_start(out=gch, in_=g_r[:, :, nsl])
            nc.sync.dma_start(out=xch, in_=x_r[:, :, nsl])
            nc.sync.dma_start(out=cch, in_=c_r[:, :, nsl])

            zg = psum.tile([128, 512], F32, tag="zg", name="zg")
            zx = psum.tile([128, 512], F32, tag="zx", name="zx")
            ph = psum.tile([128, 512], F32, tag="ph", name="ph")
            for k in range(2):
                nc.tensor.matmul(out=zg, lhsT=gw_t[k][:, osl], rhs=gch[:, k, :],
                                 start=(k == 0), stop=(k == 1))
            for k in range(4):
                nc.tensor.matmul(out=zx, lhsT=xw_t[k][:, osl], rhs=xch[:, k, :],
                                 start=(k == 0), stop=(k == 3))
            for k in range(2):
                nc.tensor.matmul(out=ph, lhsT=cw_t[k][:, osl], rhs=cch[:, k, :],
                                 start=(k == 0), stop=(k == 1))

            Iv = d["I"].rearrange("p a b -> p (a b)")
            RT = tmp.tile([128, 512], FP16, tag="trelu", name="trelu")
            s.activation(out=RT, in_=zg, func=AF.Relu)
            v.tensor_tensor(out=Iv[:, nsl], in0=RT, in1=zx, op=OP.add)
            phc = ph.rearrange("p (a b) -> p a b", b=64)
            s.copy(out=d["PHI"][:, n * 8:n * 8 + 8, :], in_=phc)
            s.copy(out=d["PHIB"][:, n * 8 + 1:n * 8 + 9, 1:65], in_=phc)

        def emit_stats_head(t):
            # U + arctan of d["PHI"]; no visible state change, safe to hoist
            d = st[t]
            PHI = d["PHI"]
            U = tmp.tile([128, 64, 64], FP16, tag="tU", name="tU")
            for r0, r1 in HS:
                v._custom_dve(uarg_op, out=U[:, r0:r1, :], in0=PHI[:, r0:r1, :],
                              s0=CHEB0, s1=CHEB1, imm2=1.0)
            Ta = tmp.tile([128, 64, 64], FP16, tag="tTa", name="tTa")
            for r0, r1 in HS:
                s.activation(out=Ta[:, r0:r1, :], in_=U[:, r0:r1, :],
                             func=AF.Arctan)
            d["Ta"] = Ta

        def emit_stats(t, head_done=False):
            # region statistics of d["PHI"] (lag-1: used by next iteration)
            d = st[t]
            PHI, I_t = d["PHI"], d["I"]
            if not head_done:
                emit_stats_head(t)
            Ta = d["Ta"]
            ST4 = sm.tile([128, 4], F32, tag="stt", name="stt")
            T = tmp.tile([128, 64, 64], FP16, tag="tT", name="tT")
            Taf = Ta.rearrange("p a b -> p (a b)")
            Tf = T.rearrange("p a b -> p (a b)")
            for h, (r0, r1) in enumerate(HS):
                v._custom_dve(fix_op, out=T[:, r0:r1, :], in0=PHI[:, r0:r1, :],
                              in1=Taf[:, r0 * 64:r1 * 64],
                              s0=1.0, s1=-0.0, imm2=HPI,
                              accum_out=ST4[:, h:h + 1])
            ITD = tmp.tile([128, 64, 66], FP16, tag="tPXN", name="tITD")
            for h, (r0, r1) in enumerate(HS):
                gp.tensor_tensor(out=ITD[:, r0:r1, 0:64],
                                 in0=I_t[:, r0:r1, :],
                                 in1=T[:, r0:r1, :], op=OP.mult)
            ITS = tmp.tile([128, 64, 66], FP16, tag="tPYN", name="tITS")
            for h, (r0, r1) in enumerate(HS):
                v.tensor_scalar(out=ITS[:, r0:r1, 0:64],
                                in0=ITD[:, r0:r1, 0:64], scalar1=1.0,
                                scalar2=None, op0=OP.mult, op1=OP.add,
                                accum_out=ST4[:, 2 + h:3 + h])
            # per-partition scalar chain -> Av, Bv
            SM = sm.tile([128, 14], F32, tag="smv", name="smv")
            a1e, s1, r1, c1, a2e, s2 = (SM[:, i:i + 1] for i in range(6))
            r2, c2, dd, ss, Avv, Bvv = (SM[:, i:i + 1] for i in range(6, 12))
            St, Sit = SM[:, 12:13], SM[:, 13:14]
            v.tensor_tensor(out=St, in0=ST4[:, 0:1], in1=ST4[:, 1:2], op=OP.add)
            v.tensor_tensor(out=Sit, in0=ST4[:, 2:3], in1=ST4[:, 3:4], op=OP.add)
            v.tensor_scalar(out=a1e, in0=St, scalar1=INV_PI,
                            scalar2=2048.0 + 1e-8, op0=OP.mult, op1=OP.add)
            v.scalar_tensor_tensor(out=s1, in0=Sit, scalar=INV_PI,
                                   in1=d["halfsI"], op0=OP.mult, op1=OP.add)
            v.reciprocal(out=r1, in_=a1e)
            v.tensor_tensor(out=c1, in0=s1, in1=r1, op=OP.mult)
            v.tensor_scalar(out=a2e, in0=a1e, scalar1=-1.0,
                            scalar2=4096.0 + 2e-8, op0=OP.mult, op1=OP.add)
            v.tensor_tensor(out=s2, in0=d["sI"], in1=s1, op=OP.subtract)
            v.reciprocal(out=r2, in_=a2e)
            v.tensor_tensor(out=c2, in0=s2, in1=r2, op=OP.mult)
            v.tensor_tensor(out=dd, in0=c1, in1=c2, op=OP.subtract)
            v.tensor_tensor(out=ss, in0=c1, in1=c2, op=OP.add)
            v.tensor_scalar(out=Avv, in0=dd, scalar1=A_SC, scalar2=None,
                            op0=OP.mult)
            v.scalar_tensor_tensor(out=Bvv, in0=Avv, scalar=-0.5,
                                   in1=ss, op0=OP.mult, op1=OP.mult)
            d["Av"], d["Bv"] = Avv, Bvv

        def finish_gemm(t):
            d = st[t]
            ISUMD = tmp.tile([128, 64, 64], FP16, tag="tU", name="tISUM")
            v.tensor_scalar(out=ISUMD, in0=d["I"], scalar1=1.0, scalar2=None,
                            op0=OP.mult, op1=OP.add, accum_out=d["sI"])
            v.tensor_scalar(out=d["halfsI"], in0=d["sI"], scalar1=0.5,
                            scalar2=None, op0=OP.mult)
            emit_stats(t)

        def pde_prologue(t, it):
            # ghost rim + central diffs + Yhat refresh for (t, it); stores
            # handles in st[t] so pde_iter can be prefetched across tiles
            d = st[t]
            PHI, PHIB, I_t = d["PHI"], d["PHIB"], d["I"]

            # ---- ghost rim of PHIB (DVE smalls, row-split for wavefront)
            for r0, r1 in HS:
                ghost_edge(v, PHIB[:, 1 + r0:1 + r1, 0],
                           PHIB[:, 1 + r0:1 + r1, 1], PHIB[:, 1 + r0:1 + r1, 2])
                ghost_edge(v, PHIB[:, 1 + r0:1 + r1, 65],
                           PHIB[:, 1 + r0:1 + r1, 64], PHIB[:, 1 + r0:1 + r1, 63])
            ghost_edge(v, PHIB[:, 0, 1:65], PHIB[:, 1, 1:65], PHIB[:, 2, 1:65])
            ghost_edge(v, PHIB[:, 65, 1:65], PHIB[:, 64, 1:65],
                       PHIB[:, 63, 1:65])

            # ---- unscaled central differences (DVE)
            PX = tmp.tile([128, 64, 64], FP16, tag="tPX", name="tPX")
            for r0, r1 in HS:
                v.tensor_tensor(out=PX[:, r0:r1, :],
                                in0=PHIB[:, 1 + r0:1 + r1, 2:66],
                                in1=PHIB[:, 1 + r0:1 + r1, 0:64],
                                op=OP.subtract)
            PY = tmp.tile([128, 64, 64], FP16, tag="tPY", name="tPY")
            for r0, r1 in HS:
                gp.tensor_tensor(out=PY[:, r0:r1, :],
                                 in0=PHIB[:, 2 + r0:2 + r1, 1:65],
                                 in1=PHIB[:, r0:r1, 1:65],
                                 op=OP.subtract)

            # ---- iteration-0 Yhat bootstrap (ESUM seed + clamped NR)
            if d["Yhat"] is None:
                S = tmp.tile([128, 64, 64], FP16, tag="tS", name="tS")
                PYf = PY.rearrange("p a b -> p (a b)")
                for r0, r1 in HS:
                    v._custom_dve(sq2_op, out=S[:, r0:r1, :],
                                  in0=PX[:, r0:r1, :],
                                  in1=PYf[:, r0 * 64:r1 * 64], s0=EPS4)
                EE = tmp.tile([128, 64, 64], FP16, tag="tT", name="tEE")
                for r0, r1 in HS:
                    v._custom_dve(esum_op, out=EE[:, r0:r1, :],
                                  in0=PX[:, r0:r1, :],
                                  in1=PYf[:, r0 * 64:r1 * 64],
                                  s0=float(1.19 * np.sqrt(2.0)), s1=4e-4)
                Yh = tmp.tile([128, 64, 64], FP16, tag="tY", name="tY",
                              bufs=2)
                for r0, r1 in HS:
                    v._custom_dve(recipf_op, out=Yh[:, r0:r1, :],
                                  in0=EE[:, r0:r1, :],
                                  s0=CHEB0, s1=CHEB1, imm2=2.0)
                d["Yhat"] = Yh
                for _ in range(1):
                    Yn = tmp.tile([128, 64, 64], FP16, tag="tY", name="tY",
                                  bufs=2)
                    Yf = d["Yhat"].rearrange("p a b -> p (a b)")
                    for r0, r1 in HS:
                        v._custom_dve(nr_op, out=Yn[:, r0:r1, :],
                                      in0=S[:, r0:r1, :],
                                      in1=Yf[:, r0 * 64:r1 * 64],
                                      s0=1.5, s1=0.25)
                    d["Yhat"] = Yn
            d["PX"], d["PY"] = PX, PY

        def pde_iter(t, it, prologue_done=False, inject=None):
            d = st[t]
            PHI, PHIB, I_t = d["PHI"], d["PHIB"], d["I"]
            do_stats = it < NITER - 1 and (STATS_EVERY_ITER or (it % 2 == 0 and it > 0))
            if not prologue_done:
                pde_prologue(t, it)
            PX, PY = d["PX"], d["PY"]
            Yhat = d["Yhat"]

            # ---- force scale/bias from (lag-1) stats (Act: Av*I + Bv)
            F0 = tmp.tile([128, 64, 64], FP16, tag="tF0", name="tF0")
            for r0, r1 in HS:
                s.activation(out=F0[:, r0:r1, :], in_=I_t[:, r0:r1, :],
                             func=AF.Identity, scale=d["Av"], bias=d["Bv"])

            # ---- normalized gradient + divergence (Pool), padded buffers
            GSC = tmp.tile([128, 4, 64], FP16, tag="tgh", name="tgh")
            PXN = tmp.tile([128, 64, 66], FP16, tag="tPXN", name="tPXN")
            for h, (r0, r1) in enumerate(HS):
                gp.tensor_tensor(out=PXN[:, r0:r1, 1:65], in0=PX[:, r0:r1, :],
                                 in1=Yhat[:, r0:r1, :], op=OP.mult)
                ghost_edge_pool(PXN[:, r0:r1, 0], PXN[:, r0:r1, 1],
                                PXN[:, r0:r1, 2], GSC[:, h, 0:32])
                ghost_edge_pool(PXN[:, r0:r1, 65], PXN[:, r0:r1, 64],
                                PXN[:, r0:r1, 63], GSC[:, h, 32:64])
            PYN = tmp.tile([128, 66, 64], FP16, tag="tPYN", name="tPYN")
            for r0, r1 in HS:
                gp.tensor_tensor(out=PYN[:, 1 + r0:1 + r1, :],
                                 in0=PY[:, r0:r1, :],
                                 in1=Yhat[:, r0:r1, :], op=OP.mult)
            ghost_edge_pool(PYN[:, 0, :], PYN[:, 1, :], PYN[:, 2, :],
                            GSC[:, 2, :])
            ghost_edge_pool(PYN[:, 65, :], PYN[:, 64, :], PYN[:, 63, :],
                            GSC[:, 3, :])

            CW = tmp.tile([128, 64, 64], FP16, tag="tPX", name="tCW")
            for r0, r1 in HS:
                v.tensor_tensor(out=CW[:, r0:r1, :],
                                in0=PXN[:, r0:r1, 2:66],
                                in1=PXN[:, r0:r1, 0:64], op=OP.subtract)
            CH = tmp.tile([128, 64, 64], FP16, tag="tPY", name="tCH")
            for r0, r1 in HS:
                gp.tensor_tensor(out=CH[:, r0:r1, :],
                                 in0=PYN[:, 2 + r0:2 + r1, :],
                                 in1=PYN[:, r0:r1, :], op=OP.subtract)
            FF = tmp.tile([128, 64, 64], FP16, tag="tFF", name="tFF")
            for r0, r1 in HS:
                gp.tensor_tensor(out=FF[:, r0:r1, :], in0=F0[:, r0:r1, :],
                                 in1=CW[:, r0:r1, :], op=OP.add)
            FF2 = tmp.tile([128, 64, 64], FP16, tag="tFF2", name="tFF2")
            for r0, r1 in HS:
                gp.tensor_tensor(out=FF2[:, r0:r1, :], in0=FF[:, r0:r1, :],
                                 in1=CH[:, r0:r1, :], op=OP.add)

            # ---- stats for the NEXT iteration (fills DVE while Pool runs)
            if do_stats:
                emit_stats(t)

            # ---- lagged Yhat refresh from THIS iteration's px/py: consumed
            # starting next iteration (keeps pxn off the S/NR chain)
            if (it % 2 == 1 and it < NITER - 1) or it == 0:
                S = tmp.tile([128, 64, 64], FP16, tag="tS", name="tS")
                PYf = PY.rearrange("p a b -> p (a b)")
                for r0, r1 in HS:
                    v._custom_dve(sq2_op, out=S[:, r0:r1, :],
                                  in0=PX[:, r0:r1, :],
                                  in1=PYf[:, r0 * 64:r1 * 64], s0=EPS4)
                Yn = tmp.tile([128, 64, 64], FP16, tag="tY", name="tY",
                              bufs=2)
                Yf = d["Yhat"].rearrange("p a b -> p (a b)")
                for r0, r1 in HS:
                    v._custom_dve(nr_op, out=Yn[:, r0:r1, :],
                                  in0=S[:, r0:r1, :],
                                  in1=Yf[:, r0 * 64:r1 * 64],
                                  s0=1.5, s1=0.25)
                d["Yhat"] = Yn

            # ---- injected cross-tile work (fills the FF2 wait)
            if inject is not None:
                inject()

            # ---- G = kappa * FF2 / (phi^2 + 1); phi' = phi + G (DVE)
            G = tmp.tile([128, 64, 64], FP16, tag="tG", name="tG")
            FF2f = FF2.rearrange("p a b -> p (a b)")
            for r0, r1 in HS:
                v._custom_dve(rff_op, out=G[:, r0:r1, :], in0=PHI[:, r0:r1, :],
                              in1=FF2f[:, r0 * 64:r1 * 64], s0=RK0, s1=RK1)
            PHI2 = state.tile([128, 64, 64], FP16, tag=f"phi{t % 2}",
                              name=f"phi2_{t}")
            for r0, r1 in HS:
                gp.tensor_tensor(out=PHI2[:, r0:r1, :], in0=G[:, r0:r1, :],
                                 in1=PHI[:, r0:r1, :], op=OP.add)
            d["PHI"] = PHI2
            if it < NITER - 1:
                for r0, r1 in HS:
                    s.copy(out=PHIB[:, 1 + r0:1 + r1, 1:65],
                           in_=PHI2[:, r0:r1, :])

        # ---------------- schedule: GEMM(0), then PDE(t) with GEMM(t+1)
        # chunks injected into the first 8 iterations
        start_tile(0)
        for n in range(8):
            gemm_chunk(0, n)
        finish_gemm(0)
        for t in range(4):
            for it in range(NITER):
                if t < 3:
                    if it == 0:
                        start_tile(t + 1)
                    if it < 8:
                        inj = (lambda tt=t + 1, nn=it: gemm_chunk(tt, nn))
                    elif it == 8:
                        inj = (lambda tt=t + 1: finish_gemm(tt))
                    else:
                        inj = (lambda tt=t + 1: pde_prologue(tt, 0))
                else:
                    inj = None
                pde_iter(t, it, prologue_done=(it == 0 and t > 0),
                         inject=inj)
            # sigmoid + writeback (f32), chunked
            PHIv = st[t]["PHI"].rearrange("p a b -> p (a b)")
            osl = slice(t * 128, (t + 1) * 128)
            for q in range(4):
                qsl = slice(q * 1024, (q + 1) * 1024)
                OUT = tmp.tile([128, 1024], FP16, tag="outq", name="outq")
                s.activation(out=OUT, in_=PHIv[:, qsl], func=AF.Sigmoid)
                nc.sync.dma_start(out=out_d[osl, qsl], in_=OUT)

    nc.compile()
    return nc


# ---------------------------------------------------------------- host entry

_CACHE = {}
LAST = {"exec_time_ns": None}


def _get_nc(dt_val, lam_val):
    key = (round(dt_val, 9), round(lam_val, 9))
    if key not in _CACHE:
        _CACHE[key] = build(dt_val, lam_val)
    return _CACHE[key]


def kernel(contour, g, x, dt, lambda_, g_w, x_w, c_w, trace=False):
    from concourse.bass_utils import run_bass_kernel_spmd

    import ml_dtypes
    BF = ml_dtypes.bfloat16
    contour = np.asarray(contour, dtype=np.float32).astype(BF)
    g = np.asarray(g, dtype=np.float32).astype(BF)
    x = np.asarray(x, dtype=np.float32).astype(BF)
    g_wT = np.ascontiguousarray(np.asarray(g_w, dtype=np.float32).T).astype(BF)
    x_wT = np.ascontiguousarray(np.asarray(x_w, dtype=np.float32).T).astype(BF)
    c_wT = np.ascontiguousarray(np.asarray(c_w, dtype=np.float32).T).astype(BF)
    dt_val = float(np.asarray(dt).reshape(-1)[0])
    lam_val = float(np.asarray(lambda_).reshape(-1)[0])

    nc = _get_nc(dt_val, lam_val)
    B = contour.shape[0]
    in_maps = []
    for b in range(B):
        in_maps.append({
            "g": np.ascontiguousarray(g[b].reshape(256, 4096)),
            "x": np.ascontiguousarray(x[b].reshape(512, 4096)),
            "contour": np.ascontiguousarray(contour[b].reshape(256, 4096)),
            "g_wT": g_wT, "x_wT": x_wT, "c_wT": c_wT,
        })
    try:
        res = run_bass_kernel_spmd(nc, in_maps, core_ids=list(range(B)),
                                   trace=trace)
    except ModuleNotFoundError:
        res = run_bass_kernel_spmd(nc, in_maps, core_ids=list(range(B)),
                                   trace=False)
    LAST["exec_time_ns"] = res.exec_time_ns
    out = np.stack([res.results[b]["out"].reshape(512, 64, 64)
                    for b in range(B)])
    return out.astype(np.float32)


# revision 29
# speedup vs baseline: 1.0517x; 1.0517x over previous
"""Trainium2 Bass kernel for nn_ACMAttention (Chan-Vese PDE attention block).

Self-contained: shards batch B=8 across 8 NeuronCores (pure data parallel).
Per core: 1x1-conv GEMMs on the TensorEngine, then a 10-iteration Chan-Vese
PDE loop in fp16 split across DVE (diffs, customs) and GPSIMD (normalized
gradient + divergence) with the Activation engine doing arctan + mirror
copies.  All big ops are emitted as row-halves so the two engines wavefront
within each iteration; the next tile's GEMM chunks are injected into the
current tile's PDE loop to avoid inter-tile bubbles.

Shapes (hardcoded, per core = one batch element):
  g [256,4096], x [512,4096], contour [256,4096], weights transposed on host.
"""
import sys
import numpy as np

for _p in ("/opt/trn_rl_repo",):
    if _p not in sys.path:
        sys.path.insert(0, _p)

from contextlib import ExitStack

import concourse.bass as bass
import concourse.bacc as bacc
import concourse.tile as tile
from concourse import mybir
from concourse import dve_ops as dvo
from concourse.dve_spec import (
    Spec, Src0, Src1, C0, C1, C2, Bin, AluOp, sq, maxx, minn, lower,
    _has_src1, select, Zero, One,
)
from concourse.dve_uop import DveOpSpec
from concourse import tile_utils

tile_utils.max_sbuf_usage = 204 * 1024

F32 = mybir.dt.float32
BF16 = mybir.dt.bfloat16
FP16 = mybir.dt.float16
AF = mybir.ActivationFunctionType
OP = mybir.AluOpType

NITER = 10
LINEARIZE = False
EPS4 = 4e-8           # 4 * 1e-8 (unscaled-gradient eps)
CHEB0 = -0.23549792   # recip NOT-seed Chebyshev pair
CHEB1 = 2.0017324

REFRESH_EVERY = 2      # refresh Yhat (rsqrt |grad|) on iters 0, k, 2k, ...
STATS_EVERY_ITER = False  # lag-1 stats every iter; False -> every other iter

HS = [(0, 32), (32, 64)]  # row-halves for wavefronting

# ---------------------------------------------------------------- custom ops

_REG = {}


def _register(name, body, reference, accum=False):
    if name in _REG:
        return _REG[name]
    row = max(dvo._SUB_OPCODE_FOR_NAME.values()) + 1
    assert row < 0x20
    dvo._SUB_OPCODE_FOR_NAME[name] = row
    if accum:
        from operator import add as _add
        spec = Spec(body=body, reference=reference, accum=_add,
                    accum_init=Zero)
    else:
        spec = Spec(body=body, reference=reference)
    shas = {}
    for ver in ("v3", "v4"):
        try:
            uops = lower(spec, ver=ver)
            shas[ver] = DveOpSpec(
                name=name, opcode=row, uops=uops, rd1_en=_has_src1(spec)
            ).sha(ver)
        except Exception:
            pass
    assert shas, f"custom op {name} failed to lower"
    op = dvo.DveOp(name, spec, subdim=False, uops_sha=shas)
    dvo.OPS.append(op)
    dvo.CUSTOM_DVE_SPECS[name] = spec
    _REG[name] = op
    return op


def _ref_sq2(in0, in1, c0, c1, c2):
    i0 = in0.astype(np.float32)
    i1 = in1.astype(np.float32).reshape(i0.shape)
    return (i0 ** 2 + i1 ** 2 + c0).astype(np.float32)


def _ref_rsqrt_nr(in0, in1, c0, c1, c2):
    i0 = in0.astype(np.float32)
    i1 = in1.astype(np.float32).reshape(i0.shape)
    m = np.maximum(np.float32(c0) - i0 * i1 * i1, np.float32(c1))
    return (i1 * m).astype(np.float32)


def _ref_atan_arg(in0, in1, c0, c1, c2):
    # U = phi * min(c2, recip1(phi^2));  DVE min drops NaN (recip1(0)=NaN -> c2)
    x = np.ascontiguousarray(in0.astype(np.float32))
    v = np.ascontiguousarray(x * x)
    nx = (~v.view(np.int32)).view(np.float32)
    y0 = nx * np.float32(c0)
    y1 = y0 * (np.float32(c1) - v * y0)
    m = np.where(np.isnan(y1), np.float32(c2), np.minimum(np.float32(c2), y1))
    return (x * m).astype(np.float32)


def _ref_atan_fix(in0, in1, c0, c1, c2):
    # in0 = phi, in1 = Ta ; out = full-range atan; accum = sum(out)
    # sp = (phi & c1) | c2 == copysign(pi/2, phi) with c1=-0.0, c2=pi/2
    phi = np.ascontiguousarray(in0.astype(np.float32))
    ta = in1.astype(np.float32).reshape(phi.shape)
    sp = ((phi.view(np.int32) & np.float32(c1).view(np.int32))
          | np.float32(c2).view(np.int32)).view(np.float32)
    b = np.where(phi * phi > np.float32(c0), sp - ta, ta).astype(np.float32)
    return b, b.reshape(b.shape[0], -1).sum(axis=-1, keepdims=True)


def _ref_esum(in0, in1, c0, c1, c2):
    i0 = in0.astype(np.float32)
    i1 = in1.astype(np.float32).reshape(i0.shape)
    return ((np.abs(i0) + np.abs(i1)) * np.float32(c0) + np.float32(c1)
            ).astype(np.float32)


def _ref_recipf(in0, in1, c0, c1, c2):
    # fp16-safe clone of RECIPROCAL_APPROX_FAST (cast before the bit trick)
    x = np.ascontiguousarray(in0.astype(np.float32))
    nx = (~x.view(np.int32)).view(np.float32)
    y0 = nx * np.float32(c0)
    y1 = y0 * (np.float32(c1) - x * y0)
    return (y1 * (np.float32(c2) - x * y1)).astype(np.float32)


def _ref_rff(in0, in1, c0, c1, c2):
    # out = cheb_recip(phi^2 + 1) * ff  with the Chebyshev pair (c0, c1)
    # pre-scaled by sqrt(kappa) so out = kappa * recip(phi^2+1) * ff
    phi = np.ascontiguousarray(in0.astype(np.float32))
    ff = in1.astype(np.float32).reshape(phi.shape)
    v = np.ascontiguousarray(phi * phi + np.float32(1.0))
    nx = (~v.view(np.int32)).view(np.float32)
    y0 = nx * np.float32(c0)
    y1 = y0 * (np.float32(c1) - v * y0)
    return (y1 * ff).astype(np.float32)


def _ref_reluadd(in0, in1, c0, c1, c2):
    i0 = in0.astype(np.float32)
    i1 = in1.astype(np.float32).reshape(i0.shape)
    return (np.maximum(i0, np.float32(0.0)) + i1).astype(np.float32)


def get_ops():
    sq2 = _register("ACM_SQ2ADDC", sq(Src0) + sq(Src1) + C0, _ref_sq2)
    nr = _register(
        "ACM_RSQRT_NR",
        Src1 * maxx(C0 - Src0 * sq(Src1), C1),
        _ref_rsqrt_nr,
    )
    _a0 = maxx(Src0, Zero - Src0)
    _a1 = maxx(Src1, Zero - Src1)
    esum = _register("ACM_ESUM", (_a0 + _a1) * C0 + C1, _ref_esum)
    _v2 = sq(Src0)
    _n = Bin(AluOp.BITWISE_NOT, _v2, _v2)
    _u0 = _n * C0
    _u1 = _u0 * (C1 - _v2 * _u0)
    uarg = _register("ACM_ATAN_ARG", Src0 * minn(C2, _u1), _ref_atan_arg)
    _sp = Bin(AluOp.BITWISE_OR, Bin(AluOp.BITWISE_AND, Src0, C1), C2)
    _tbody = select(sq(Src0) > C0, _sp - Src1, Src1)
    fix = _register("ACM_ATAN_FIX", _tbody, _ref_atan_fix, accum=True)
    _rnx = Bin(AluOp.BITWISE_NOT, Src0, Src0)
    _ry0 = _rnx * C0
    _ry1 = _ry0 * (C1 - Src0 * _ry0)
    recipf = _register("ACM_RECIPF", _ry1 * (C2 - Src0 * _ry1), _ref_recipf)
    _v = sq(Src0) + One
    _nx = Bin(AluOp.BITWISE_NOT, _v, _v)
    _y0 = _nx * C0
    _y1 = _y0 * (C1 - _v * _y0)
    rff = _register("ACM_RFF", _y1 * Src1, _ref_rff)
    reluadd = _register("ACM_RELUADD", maxx(Src0, Zero) + Src1, _ref_reluadd)
    return sq2, nr, esum, uarg, fix, recipf, rff, reluadd


# ---------------------------------------------------------------- builder

def build(dt_val: float, lam_val: float) -> bass.Bass:
    (sq2_op, nr_op, esum_op, uarg_op, fix_op, recipf_op, rff_op,
     reluadd_op) = get_ops()

    nc = bacc.Bacc()
    g_d = nc.declare_dram_parameter("g", [256, 4096], BF16, isOutput=False)
    x_d = nc.declare_dram_parameter("x", [512, 4096], BF16, isOutput=False)
    c_d = nc.declare_dram_parameter("contour", [256, 4096], BF16, isOutput=False)
    gw_d = nc.declare_dram_parameter("g_wT", [256, 512], BF16, isOutput=False)
    xw_d = nc.declare_dram_parameter("x_wT", [512, 512], BF16, isOutput=False)
    cw_d = nc.declare_dram_parameter("c_wT", [256, 512], BF16, isOutput=False)
    out_d = nc.declare_dram_parameter("out", [512, 4096], FP16, isOutput=True)

    # kappa = 1/s_r = dt/(pi*sqrt(2)); folded into the RFF Chebyshev pair
    # (scaled by sqrt(kappa)) so G = kappa * recip(phi^2+1) * FF in one op
    KAP = float(dt_val / (np.pi * np.sqrt(2.0)))
    RK0 = float(CHEB0 * np.sqrt(KAP))
    RK1 = float(CHEB1 * np.sqrt(KAP))
    A_SC = float(2.0 * np.sqrt(2.0) * lam_val)
    INV_PI = float(1.0 / np.pi)
    HPI = float(np.pi / 2)

    with tile.TileContext(nc, linearize=LINEARIZE) as tc, ExitStack() as ctx:
        v = nc.vector
        s = nc.scalar
        gp = nc.gpsimd

        wpool = ctx.enter_context(tc.tile_pool(name="w", bufs=1))
        inpool = ctx.enter_context(tc.tile_pool(name="in", bufs=2))
        psum = ctx.enter_context(tc.tile_pool(name="ps", bufs=2, space="PSUM"))
        state = ctx.enter_context(tc.tile_pool(name="st", bufs=2))
        tmp = ctx.enter_context(tc.tile_pool(name="tmp", bufs=1))
        sm = ctx.enter_context(tc.tile_pool(name="sm", bufs=2))

        # ---- load weights once (bf16, converted host-side): lhsT [k, o]
        gw_b = wpool.tile([128, 2, 512], BF16, tag="gwb", name="gwb")
        xw_b = wpool.tile([128, 4, 512], BF16, tag="xwb", name="xwb")
        cw_b = wpool.tile([128, 2, 512], BF16, tag="cwb", name="cwb")
        nc.sync.dma_start(out=gw_b, in_=gw_d.rearrange("(k p) m -> p k m", p=128))
        nc.sync.dma_start(out=xw_b, in_=xw_d.rearrange("(k p) m -> p k m", p=128))
        nc.sync.dma_start(out=cw_b, in_=cw_d.rearrange("(k p) m -> p k m", p=128))
        gw_t = [gw_b[:, k, :] for k in range(2)]
        xw_t = [xw_b[:, k, :] for k in range(4)]
        cw_t = [cw_b[:, k, :] for k in range(2)]
        # absorber matmuls: PE observes the weight DMA ticks once
        pdum = psum.tile([1, 1], F32, tag="pdum", name="pdum", bufs=1)
        nc.tensor.matmul(out=pdum, lhsT=gw_b[:, 0, 0:1], rhs=gw_b[:, 0, 0:1],
                         start=True, stop=True)
        nc.tensor.matmul(out=pdum, lhsT=xw_b[:, 0, 0:1], rhs=xw_b[:, 0, 0:1],
                         start=True, stop=True)
        nc.tensor.matmul(out=pdum, lhsT=cw_b[:, 0, 0:1], rhs=cw_b[:, 0, 0:1],
                         start=True, stop=True)

        g_r = g_d.rearrange("(k p) m -> p k m", p=128)
        x_r = x_d.rearrange("(k p) m -> p k m", p=128)
        c_r = c_d.rearrange("(k p) m -> p k m", p=128)

        def ghost_edge(eng, dst, a0, a1):
            # dst = 2*a0 - a1 (ghost extrapolation == one-sided edge diff)
            eng.scalar_tensor_tensor(out=dst, in0=a0, scalar=2.0, in1=a1,
                                     op0=OP.mult, op1=OP.subtract)

        def ghost_edge_pool(dst, a0, a1, scr):
            # Pool lacks TensorScalarPtr on HW: 2*a0 - a1 as two TTs
            gp.tensor_tensor(out=scr, in0=a0, in1=a1, op=OP.subtract)
            gp.tensor_tensor(out=dst, in0=a0, in1=scr, op=OP.add)

        st = {}

        def start_tile(t):
            p = t % 2
            d = {}
            d["PHI"] = state.tile([128, 64, 64], FP16, tag=f"phi{p}",
                                  name=f"phi_{t}")
            d["PHIB"] = state.tile([128, 66, 66], FP16, tag=f"phib{p}",
                                   name=f"phib_{t}", bufs=1)
            d["I"] = state.tile([128, 64, 64], FP16, tag=f"ii{p}",
                                name=f"ii_{t}", bufs=1)
            SIT2 = sm.tile([128, 2], F32, tag=f"sI{p}", name=f"sI_{t}")
            d["sI"] = SIT2[:, 0:1]
            d["halfsI"] = SIT2[:, 1:2]
            d["Yhat"] = None
            st[t] = d

        def gemm_chunk(t, n):
            d = st[t]
            osl = slice(t * 128, (t + 1) * 128)
            nsl = slice(n * 512, (n + 1) * 512)
            gch = inpool.tile([128, 2, 512], BF16, tag="gch", name="gch")
            xch = inpool.tile([128, 4, 512], BF16, tag="xch", name="xch")
            cch = inpool.tile([128, 2, 512], BF16, tag="cch", name="cch")
            nc.sync.dma_start(out=gch, in_=g_r[:, :, nsl])
            nc.sync.dma_start(out=xch, in_=x_r[:, :, nsl])
            nc.sync.dma_start(out=cch, in_=c_r[:, :, nsl])

            zg = psum.tile([128, 512], F32, tag="zg", name="zg")
            zx = psum.tile([128, 512], F32, tag="zx", name="zx")
            ph = psum.tile([128, 512], F32, tag="ph", name="ph")
            for k in range(2):
                nc.tensor.matmul(out=zg, lhsT=gw_t[k][:, osl], rhs=gch[:, k, :],
                                 start=(k == 0), stop=(k == 1))
            for k in range(4):
                nc.tensor.matmul(out=zx, lhsT=xw_t[k][:, osl], rhs=xch[:, k, :],
                                 start=(k == 0), stop=(k == 3))
            for k in range(2):
                nc.tensor.matmul(out=ph, lhsT=cw_t[k][:, osl], rhs=cch[:, k, :],
                                 start=(k == 0), stop=(k == 1))

            Iv = d["I"].rearrange("p a b -> p (a b)")
            RT = tmp.tile([128, 512], FP16, tag="trelu", name="trelu")
            s.activation(out=RT, in_=zg, func=AF.Relu)
            v.tensor_tensor(out=Iv[:, nsl], in0=RT, in1=zx, op=OP.add)
            phc = ph.rearrange("p (a b) -> p a b", b=64)
            s.copy(out=d["PHI"][:, n * 8:n * 8 + 8, :], in_=phc)
            s.copy(out=d["PHIB"][:, n * 8 + 1:n * 8 + 9, 1:65], in_=phc)

        def emit_stats_head(t):
            # U + arctan of d["PHI"]; no visible state change, safe to hoist
            d = st[t]
            PHI = d["PHI"]
            U = tmp.tile([128, 64, 64], FP16, tag="tU", name="tU")
            for r0, r1 in HS:
                v._custom_dve(uarg_op, out=U[:, r0:r1, :], in0=PHI[:, r0:r1, :],
                              s0=CHEB0, s1=CHEB1, imm2=1.0)
            Ta = tmp.tile([128, 64, 64], FP16, tag="tTa", name="tTa")
            for r0, r1 in HS:
                s.activation(out=Ta[:, r0:r1, :], in_=U[:, r0:r1, :],
                             func=AF.Arctan)
            d["Ta"] = Ta

        def emit_stats(t, head_done=False):
            # region statistics of d["PHI"] (lag-1: used by next iteration)
            d = st[t]
            PHI, I_t = d["PHI"], d["I"]
            if not head_done:
                emit_stats_head(t)
            Ta = d["Ta"]
            ST4 = sm.tile([128, 4], F32, tag="stt", name="stt")
            T = tmp.tile([128, 64, 64], FP16, tag="tT", name="tT")
            Taf = Ta.rearrange("p a b -> p (a b)")
            Tf = T.rearrange("p a b -> p (a b)")
            for h, (r0, r1) in enumerate(HS):
                v._custom_dve(fix_op, out=T[:, r0:r1, :], in0=PHI[:, r0:r1, :],
                              in1=Taf[:, r0 * 64:r1 * 64],
                              s0=1.0, s1=-0.0, imm2=HPI,
                              accum_out=ST4[:, h:h + 1])
            ITD = tmp.tile([128, 64, 66], FP16, tag="tPXN", name="tITD")
            for h, (r0, r1) in enumerate(HS):
                gp.tensor_tensor(out=ITD[:, r0:r1, 0:64],
                                 in0=I_t[:, r0:r1, :],
                                 in1=T[:, r0:r1, :], op=OP.mult)
            ITS = tmp.tile([128, 64, 66], FP16, tag="tPYN", name="tITS")
            for h, (r0, r1) in enumerate(HS):
                v.tensor_scalar(out=ITS[:, r0:r1, 0:64],
                                in0=ITD[:, r0:r1, 0:64], scalar1=1.0,
                                scalar2=None, op0=OP.mult, op1=OP.add,
                                accum_out=ST4[:, 2 + h:3 + h])
            # per-partition scalar chain -> Av, Bv
            SM = sm.tile([128, 14], F32, tag="smv", name="smv")
            a1e, s1, r1, c1, a2e, s2 = (SM[:, i:i + 1] for i in range(6))
            r2, c2, dd, ss, Avv, Bvv = (SM[:, i:i + 1] for i in range(6, 12))
            St, Sit = SM[:, 12:13], SM[:, 13:14]
            v.tensor_tensor(out=St, in0=ST4[:, 0:1], in1=ST4[:, 1:2], op=OP.add)
            v.tensor_tensor(out=Sit, in0=ST4[:, 2:3], in1=ST4[:, 3:4], op=OP.add)
            v.tensor_scalar(out=a1e, in0=St, scalar1=INV_PI,
                            scalar2=2048.0 + 1e-8, op0=OP.mult, op1=OP.add)
            v.scalar_tensor_tensor(out=s1, in0=Sit, scalar=INV_PI,
                                   in1=d["halfsI"], op0=OP.mult, op1=OP.add)
            v.reciprocal(out=r1, in_=a1e)
            v.tensor_tensor(out=c1, in0=s1, in1=r1, op=OP.mult)
            v.tensor_scalar(out=a2e, in0=a1e, scalar1=-1.0,
                            scalar2=4096.0 + 2e-8, op0=OP.mult, op1=OP.add)
            v.tensor_tensor(out=s2, in0=d["sI"], in1=s1, op=OP.subtract)
            v.reciprocal(out=r2, in_=a2e)
            v.tensor_tensor(out=c2, in0=s2, in1=r2, op=OP.mult)
            v.tensor_tensor(out=dd, in0=c1, in1=c2, op=OP.subtract)
            v.tensor_tensor(out=ss, in0=c1, in1=c2, op=OP.add)
            v.tensor_scalar(out=Avv, in0=dd, scalar1=A_SC, scalar2=None,
                            op0=OP.mult)
            v.scalar_tensor_tensor(out=Bvv, in0=Avv, scalar=-0.5,
                                   in1=ss, op0=OP.mult, op1=OP.mult)
            d["Av"], d["Bv"] = Avv, Bvv

        def finish_gemm(t):
            d = st[t]
            ISUMD = tmp.tile([128, 64, 64], FP16, tag="tU", name="tISUM")
            v.tensor_scalar(out=ISUMD, in0=d["I"], scalar1=1.0, scalar2=None,
                            op0=OP.mult, op1=OP.add, accum_out=d["sI"])
            v.tensor_scalar(out=d["halfsI"], in0=d["sI"], scalar1=0.5,
                            scalar2=None, op0=OP.mult)
            emit_stats(t)

        def pde_prologue(t, it):
            # ghost rim + central diffs + Yhat refresh for (t, it); stores
            # handles in st[t] so pde_iter can be prefetched across tiles
            d = st[t]
            PHI, PHIB, I_t = d["PHI"], d["PHIB"], d["I"]

            # ---- ghost rim of PHIB (DVE smalls, row-split for wavefront)
            for r0, r1 in HS:
                ghost_edge(v, PHIB[:, 1 + r0:1 + r1, 0],
                           PHIB[:, 1 + r0:1 + r1, 1], PHIB[:, 1 + r0:1 + r1, 2])
                ghost_edge(v, PHIB[:, 1 + r0:1 + r1, 65],
                           PHIB[:, 1 + r0:1 + r1, 64], PHIB[:, 1 + r0:1 + r1, 63])
            ghost_edge(v, PHIB[:, 0, 1:65], PHIB[:, 1, 1:65], PHIB[:, 2, 1:65])
            ghost_edge(v, PHIB[:, 65, 1:65], PHIB[:, 64, 1:65],
                       PHIB[:, 63, 1:65])

            # ---- unscaled central differences (DVE)
            PX = tmp.tile([128, 64, 64], FP16, tag="tPX", name="tPX")
            for r0, r1 in HS:
                v.tensor_tensor(out=PX[:, r0:r1, :],
                                in0=PHIB[:, 1 + r0:1 + r1, 2:66],
                                in1=PHIB[:, 1 + r0:1 + r1, 0:64],
                                op=OP.subtract)
            PY = tmp.tile([128, 64, 64], FP16, tag="tPY", name="tPY")
            for r0, r1 in HS:
                v.tensor_tensor(out=PY[:, r0:r1, :],
                                in0=PHIB[:, 2 + r0:2 + r1, 1:65],
                                in1=PHIB[:, r0:r1, 1:65],
                                op=OP.subtract)

            # ---- iteration-0 Yhat bootstrap (ESUM seed + clamped NR)
            if d["Yhat"] is None:
                S = tmp.tile([128, 64, 64], FP16, tag="tS", name="tS")
                PYf = PY.rearrange("p a b -> p (a b)")
                for r0, r1 in HS:
                    v._custom_dve(sq2_op, out=S[:, r0:r1, :],
                                  in0=PX[:, r0:r1, :],
                                  in1=PYf[:, r0 * 64:r1 * 64], s0=EPS4)
                EE = tmp.tile([128, 64, 64], FP16, tag="tT", name="tEE")
                for r0, r1 in HS:
                    v._custom_dve(esum_op, out=EE[:, r0:r1, :],
                                  in0=PX[:, r0:r1, :],
                                  in1=PYf[:, r0 * 64:r1 * 64],
                                  s0=float(1.19 * np.sqrt(2.0)), s1=4e-4)
                Yh = tmp.tile([128, 64, 64], FP16, tag="tY", name="tY",
                              bufs=2)
                for r0, r1 in HS:
                    v._custom_dve(recipf_op, out=Yh[:, r0:r1, :],
                                  in0=EE[:, r0:r1, :],
                                  s0=CHEB0, s1=CHEB1, imm2=2.0)
                d["Yhat"] = Yh
                for _ in range(1):
                    Yn = tmp.tile([128, 64, 64], FP16, tag="tY", name="tY",
                                  bufs=2)
                    Yf = d["Yhat"].rearrange("p a b -> p (a b)")
                    for r0, r1 in HS:
                        v._custom_dve(nr_op, out=Yn[:, r0:r1, :],
                                      in0=S[:, r0:r1, :],
                                      in1=Yf[:, r0 * 64:r1 * 64],
                                      s0=1.5, s1=0.25)
                    d["Yhat"] = Yn
            d["PX"], d["PY"] = PX, PY

        def pde_iter(t, it, prologue_done=False, inject=None):
            d = st[t]
            PHI, PHIB, I_t = d["PHI"], d["PHIB"], d["I"]
            do_stats = it < NITER - 1 and (STATS_EVERY_ITER or (it % 2 == 0 and it > 0))
            if not prologue_done:
                pde_prologue(t, it)
            PX, PY = d["PX"], d["PY"]
            Yhat = d["Yhat"]

            # ---- force scale/bias from (lag-1) stats (Act: Av*I + Bv)
            F0 = tmp.tile([128, 64, 64], FP16, tag="tF0", name="tF0")
            for r0, r1 in HS:
                s.activation(out=F0[:, r0:r1, :], in_=I_t[:, r0:r1, :],
                             func=AF.Identity, scale=d["Av"], bias=d["Bv"])

            # ---- normalized gradient + divergence (Pool), padded buffers
            GSC = tmp.tile([128, 4, 64], FP16, tag="tgh", name="tgh")
            PXN = tmp.tile([128, 64, 66], FP16, tag="tPXN", name="tPXN")
            for h, (r0, r1) in enumerate(HS):
                gp.tensor_tensor(out=PXN[:, r0:r1, 1:65], in0=PX[:, r0:r1, :],
                                 in1=Yhat[:, r0:r1, :], op=OP.mult)
                ghost_edge_pool(PXN[:, r0:r1, 0], PXN[:, r0:r1, 1],
                                PXN[:, r0:r1, 2], GSC[:, h, 0:32])
                ghost_edge_pool(PXN[:, r0:r1, 65], PXN[:, r0:r1, 64],
                                PXN[:, r0:r1, 63], GSC[:, h, 32:64])
            PYN = tmp.tile([128, 66, 64], FP16, tag="tPYN", name="tPYN")
            for r0, r1 in HS:
                gp.tensor_tensor(out=PYN[:, 1 + r0:1 + r1, :],
                                 in0=PY[:, r0:r1, :],
                                 in1=Yhat[:, r0:r1, :], op=OP.mult)
            ghost_edge_pool(PYN[:, 0, :], PYN[:, 1, :], PYN[:, 2, :],
                            GSC[:, 2, :])
            ghost_edge_pool(PYN[:, 65, :], PYN[:, 64, :], PYN[:, 63, :],
                            GSC[:, 3, :])

            CW = tmp.tile([128, 64, 64], FP16, tag="tPX", name="tCW")
            for r0, r1 in HS:
                v.tensor_tensor(out=CW[:, r0:r1, :],
                                in0=PXN[:, r0:r1, 2:66],
                                in1=PXN[:, r0:r1, 0:64], op=OP.subtract)
            CH = tmp.tile([128, 64, 64], FP16, tag="tPY", name="tCH")
            for r0, r1 in HS:
                gp.tensor_tensor(out=CH[:, r0:r1, :],
                                 in0=PYN[:, 2 + r0:2 + r1, :],
                                 in1=PYN[:, r0:r1, :], op=OP.subtract)
            FF = tmp.tile([128, 64, 64], FP16, tag="tFF", name="tFF")
            for r0, r1 in HS:
                gp.tensor_tensor(out=FF[:, r0:r1, :], in0=F0[:, r0:r1, :],
                                 in1=CW[:, r0:r1, :], op=OP.add)
            FF2 = tmp.tile([128, 64, 64], FP16, tag="tFF2", name="tFF2")
            for r0, r1 in HS:
                gp.tensor_tensor(out=FF2[:, r0:r1, :], in0=FF[:, r0:r1, :],
                                 in1=CH[:, r0:r1, :], op=OP.add)

            # ---- stats for the NEXT iteration (fills DVE while Pool runs)
            if do_stats:
                emit_stats(t)

            # ---- lagged Yhat refresh from THIS iteration's px/py: consumed
            # starting next iteration (keeps pxn off the S/NR chain)
            if (it % 2 == 1 and it < NITER - 1) or it == 0:
                S = tmp.tile([128, 64, 64], FP16, tag="tS", name="tS")
                PYf = PY.rearrange("p a b -> p (a b)")
                for r0, r1 in HS:
                    v._custom_dve(sq2_op, out=S[:, r0:r1, :],
                                  in0=PX[:, r0:r1, :],
                                  in1=PYf[:, r0 * 64:r1 * 64], s0=EPS4)
                Yn = tmp.tile([128, 64, 64], FP16, tag="tY", name="tY",
                              bufs=2)
                Yf = d["Yhat"].rearrange("p a b -> p (a b)")
                for r0, r1 in HS:
                    v._custom_dve(nr_op, out=Yn[:, r0:r1, :],
                                  in0=S[:, r0:r1, :],
                                  in1=Yf[:, r0 * 64:r1 * 64],
                                  s0=1.5, s1=0.25)
                d["Yhat"] = Yn

            # ---- injected cross-tile work (fills the FF2 wait)
            if inject is not None:
                inject()

            # ---- G = kappa * FF2 / (phi^2 + 1); phi' = phi + G (DVE)
            G = tmp.tile([128, 64, 64], FP16, tag="tG", name="tG")
            FF2f = FF2.rearrange("p a b -> p (a b)")
            for r0, r1 in HS:
                v._custom_dve(rff_op, out=G[:, r0:r1, :], in0=PHI[:, r0:r1, :],
                              in1=FF2f[:, r0 * 64:r1 * 64], s0=RK0, s1=RK1)
            PHI2 = state.tile([128, 64, 64], FP16, tag=f"phi{t % 2}",
                              name=f"phi2_{t}")
            for r0, r1 in HS:
                gp.tensor_tensor(out=PHI2[:, r0:r1, :], in0=G[:, r0:r1, :],
                                 in1=PHI[:, r0:r1, :], op=OP.add)
            d["PHI"] = PHI2
            if it < NITER - 1:
                for r0, r1 in HS:
                    s.copy(out=PHIB[:, 1 + r0:1 + r1, 1:65],
                           in_=PHI2[:, r0:r1, :])

        # ---------------- schedule: GEMM(0), then PDE(t) with GEMM(t+1)
        # chunks injected into the first 8 iterations
        start_tile(0)
        for n in range(8):
            gemm_chunk(0, n)
        finish_gemm(0)
        for t in range(4):
            for it in range(NITER):
                if t < 3:
                    if it == 0:
                        start_tile(t + 1)
                    if it < 8:
                        inj = (lambda tt=t + 1, nn=it: gemm_chunk(tt, nn))
                    elif it == 8:
                        inj = (lambda tt=t + 1: finish_gemm(tt))
                    else:
                        inj = (lambda tt=t + 1: pde_prologue(tt, 0))
                else:
                    inj = None
                pde_iter(t, it, prologue_done=(it == 0 and t > 0),
                         inject=inj)
            # sigmoid + writeback (f32), chunked
            PHIv = st[t]["PHI"].rearrange("p a b -> p (a b)")
            osl = slice(t * 128, (t + 1) * 128)
            for q in range(4):
                qsl = slice(q * 1024, (q + 1) * 1024)
                OUT = tmp.tile([128, 1024], FP16, tag="outq", name="outq")
                s.activation(out=OUT, in_=PHIv[:, qsl], func=AF.Sigmoid)
                nc.sync.dma_start(out=out_d[osl, qsl], in_=OUT)

    nc.compile()
    return nc


# ---------------------------------------------------------------- host entry

_CACHE = {}
LAST = {"exec_time_ns": None}


def _get_nc(dt_val, lam_val):
    key = (round(dt_val, 9), round(lam_val, 9))
    if key not in _CACHE:
        _CACHE[key] = build(dt_val, lam_val)
    return _CACHE[key]


def kernel(contour, g, x, dt, lambda_, g_w, x_w, c_w, trace=False):
    from concourse.bass_utils import run_bass_kernel_spmd

    import ml_dtypes
    BF = ml_dtypes.bfloat16
    contour = np.asarray(contour, dtype=np.float32).astype(BF)
    g = np.asarray(g, dtype=np.float32).astype(BF)
    x = np.asarray(x, dtype=np.float32).astype(BF)
    g_wT = np.ascontiguousarray(np.asarray(g_w, dtype=np.float32).T).astype(BF)
    x_wT = np.ascontiguousarray(np.asarray(x_w, dtype=np.float32).T).astype(BF)
    c_wT = np.ascontiguousarray(np.asarray(c_w, dtype=np.float32).T).astype(BF)
    dt_val = float(np.asarray(dt).reshape(-1)[0])
    lam_val = float(np.asarray(lambda_).reshape(-1)[0])

    nc = _get_nc(dt_val, lam_val)
    B = contour.shape[0]
    in_maps = []
    for b in range(B):
        in_maps.append({
            "g": np.ascontiguousarray(g[b].reshape(256, 4096)),
            "x": np.ascontiguousarray(x[b].reshape(512, 4096)),
            "contour": np.ascontiguousarray(contour[b].reshape(256, 4096)),
            "g_wT": g_wT, "x_wT": x_wT, "c_wT": c_wT,
        })
    try:
        res = run_bass_kernel_spmd(nc, in_maps, core_ids=list(range(B)),
                                   trace=trace)
    except ModuleNotFoundError:
        res = run_bass_kernel_spmd(nc, in_maps, core_ids=list(range(B)),
                                   trace=False)
    LAST["exec_time_ns"] = res.exec_time_ns
    out = np.stack([res.results[b]["out"].reshape(512, 64, 64)
                    for b in range(B)])
    return out.astype(np.float32)
